# revision 1
# baseline (speedup 1.0000x reference)
"""GAT node encoder (3 GATConv+BN layers) on 8 trn2 NeuronCores.

Sharding: nodes partitioned across cores (dst-sharded message passing).
Per layer, per core:
  1. local matmul of this core's node shard: [h | s | d] = y @ [W | W@a_src | W@a_dst]
  2. AllGather of the [h | s] node table (node-major rows) across cores
  3. per dst-tile (128 nodes, degree-sorted ELL layout): indirect-DMA row
     gathers of h[src], flash-style segment softmax over incoming edges,
     weighted accumulation, head mean
  4. BatchNorm: feature-major stats via free-axis reduction + AllReduce of
     per-feature sums, fused scale/shift(+ReLU) activation.

The per-feature bias b is dropped: BN(o + b) == BN(o) exactly (b shifts every
node's feature equally, so it cancels in mean subtraction and leaves var
unchanged).
"""
import os
import sys

sys.path.insert(0, "/opt/trn_rl_repo")

import numpy as np

import concourse.bass as bass
import concourse.bacc as bacc
import concourse.tile as tile
from concourse import mybir
from concourse import bass_utils
from concourse.masks import make_identity

NCORES = 8
P = 128
NEG_SLOPE = 0.2
EPS_BN = 1e-5
CHUNK = 12  # ELL slots processed per flash-softmax chunk

F32 = mybir.dt.float32
I32 = mybir.dt.int32


# ----------------------------------------------------------------------------
# host-side graph preprocessing
# ----------------------------------------------------------------------------

def _prep(edge_index, N):
    src = np.asarray(edge_index[0], dtype=np.int64)
    dst = np.asarray(edge_index[1], dtype=np.int64)
    loops = np.arange(N, dtype=np.int64)
    src = np.concatenate([src, loops])
    dst = np.concatenate([dst, loops])

    shard = N // NCORES                      # real nodes per core
    ntiles = (shard + P) // P                # always >= 1 pad row per shard
    shard_pad = ntiles * P                   # padded rows per core shard
    pad_row = shard                          # global table row of a guaranteed pad node (rank 0)

    # per-core node permutation (degree-descending) + global row ids
    deg = np.bincount(dst, minlength=N)
    node_row = np.empty(N, np.int64)         # orig node -> global table row
    core_nodes = []                          # core -> orig node id per local row (len shard_pad, -1 pad)
    for c in range(NCORES):
        lo = c * shard
        nodes = np.arange(lo, lo + shard)
        order = np.argsort(-deg[lo:lo + shard], kind="stable")
        nodes = nodes[order]
        node_row[nodes] = c * shard_pad + np.arange(shard)
        padded = np.full(shard_pad, -1, np.int64)
        padded[:shard] = nodes
        core_nodes.append(padded)

    # per-tile slot widths S_t (max over cores so the SPMD program is uniform)
    # and per-core slot index arrays
    S = np.zeros(ntiles, np.int64)
    per_core = []
    for c in range(NCORES):
        nodes = core_nodes[c]
        degs = np.where(nodes >= 0, deg[np.maximum(nodes, 0)], 0)
        S = np.maximum(S, degs.reshape(ntiles, P).max(axis=1))
        per_core.append(degs)
    S = np.maximum(S, 1)

    offs = np.zeros(ntiles + 1, np.int64)
    offs[1:] = np.cumsum(S)
    stot = int(offs[-1])

    # build slot index arrays: idx[core][p, off_t + j] = table row of src (pad_row for empty)
    idx = np.full((NCORES, P, stot), pad_row, np.int32)
    # bucket edges by (core, local dst row)
    local = node_row[dst]                    # dst's global row
    c_of = local // shard_pad
    r_of = local % shard_pad
    order = np.lexsort((r_of, c_of))
    src_s, c_s, r_s = node_row[src][order], c_of[order], r_of[order]
    # within each (c, r) run, slot j = running position
    boundaries = np.flatnonzero(np.r_[True, (c_s[1:] != c_s[:-1]) | (r_s[1:] != r_s[:-1])])
    run_id = np.zeros(len(c_s), np.int64)
    run_id[boundaries] = 1
    run_id = np.cumsum(run_id) - 1
    j_in_run = np.arange(len(c_s)) - boundaries[run_id]
    t_s = r_s // P
    p_s = r_s % P
    idx[c_s, p_s, offs[t_s] + j_in_run] = src_s.astype(np.int32)

    out_of_core = [core_nodes[c][:shard] for c in range(NCORES)]  # orig node per local row
    return {
        "shard": shard, "shard_pad": shard_pad, "ntiles": ntiles,
        "S": S.astype(int).tolist(), "offs": offs.astype(int).tolist(),
        "stot": stot, "idx": idx, "node_row": node_row,
        "out_nodes": out_of_core, "pad_row": pad_row,
    }


# ----------------------------------------------------------------------------
# device program
# ----------------------------------------------------------------------------

def _build_program(g, layers, in_dim, ablate=()):
    """layers: list of dicts {H, C, R, hs_off} per layer.
    R = table row f32 elems (h | s | pad), hs_off = offset of s in row (= H*C).
    ablate: subset of {"gather", "edge", "coll", "mm"} to skip (timing studies).
    """
    ablate = set(ablate)
    shard_pad, ntiles = g["shard_pad"], g["ntiles"]
    S, offs, stot = g["S"], g["offs"], g["stot"]
    nrows = NCORES * shard_pad
    n_l = len(layers)

    nc = bacc.Bacc("TRN2", target_bir_lowering=False, debug=False, num_devices=NCORES)

    xT = nc.dram_tensor("xT", [in_dim, shard_pad], F32, kind="ExternalInput").ap()
    idx_in = nc.dram_tensor("idx", [P, stot], I32, kind="ExternalInput").ap()
    wexts = [nc.dram_tensor(f"wext{l}", [(in_dim if l == 0 else layers[l - 1]["C"]),
                                         layers[l]["H"] * layers[l]["C"] + 2 * layers[l]["H"]],
                            F32, kind="ExternalInput").ap() for l in range(n_l)]
    gb = nc.dram_tensor("gb", [P, 2 * n_l], F32, kind="ExternalInput").ap()  # gamma|beta columns per layer
    spad_in = nc.dram_tensor("spad", [P, 1], F32, kind="ExternalInput").ap()  # 0 / -1e30 pad-row column
    out_t = nc.dram_tensor("out", [P, shard_pad], F32, kind="ExternalOutput").ap()

    with tile.TileContext(nc) as tc:
        import contextlib
        with contextlib.ExitStack() as ctx:
            dram = ctx.enter_context(tc.tile_pool(name="dram", bufs=1, space="DRAM"))
            psum = ctx.enter_context(tc.tile_pool(name="psum", bufs=2, space="PSUM"))
            psum4 = ctx.enter_context(tc.tile_pool(name="psum4", bufs=4, space="PSUM"))
            sb = ctx.enter_context(tc.tile_pool(name="sb", bufs=1))
            sb2 = ctx.enter_context(tc.tile_pool(name="sb2", bufs=2))
            sb3 = ctx.enter_context(tc.tile_pool(name="sb3", bufs=3))
            sb4 = ctx.enter_context(tc.tile_pool(name="sb4", bufs=4))
            sb6 = ctx.enter_context(tc.tile_pool(name="sb6", bufs=4))
            sb12 = ctx.enter_context(tc.tile_pool(name="sb12", bufs=12))

            ident = sb.tile([P, P], F32, tag="ident")
            make_identity(nc, ident[:])
            idx_t = sb.tile([P, stot], I32, tag="idx")
            nc.sync.dma_start(idx_t[:], idx_in[:])
            gb_t = sb.tile([P, 2 * n_l], F32, tag="gb")
            nc.sync.dma_start(gb_t[:], gb[:])
            spad_t = sb.tile([P, 1], F32, tag="spad")
            nc.sync.dma_start(spad_t[:], spad_in[:])

            yT = None  # [P(feat), shard_pad] SBUF, input to next layer (None => xT DRAM)
            for l, L in enumerate(layers):
                H, C, R, s_off = L["H"], L["C"], L["R"], L["hs_off"]
                HC = H * C
                K = in_dim if l == 0 else layers[l - 1]["C"]
                kchunks = K // P

                ag_in = dram.tile([shard_pad, R], F32, tag=f"agin{l}")
                table = dram.tile([nrows, R], F32, tag=f"table{l}")

                wk = []
                for k in range(kchunks):
                    w = sb2.tile([P, HC + 2 * H], F32, tag="wext")
                    nc.sync.dma_start(w[:], wexts[l][k * P:(k + 1) * P, :])
                    wk.append(w)

                d_buf = sb.tile([P, ntiles * H], F32, tag=f"dbuf{l % 2}")

                # ---- phase 1: local shard matmul -> hs rows + d ----
                for t in range(ntiles):
                    ph = psum.tile([P, HC], F32, tag="mmh", space="PSUM")
                    psd = psum.tile([P, 2 * H], F32, tag="mmsd", space="PSUM")
                    for k in range(kchunks if "mm" not in ablate else 0):
                        if yT is None:
                            lhsT = sb4.tile([P, P], F32, tag="xt")
                            nc.sync.dma_start(lhsT[:], xT[k * P:(k + 1) * P, t * P:(t + 1) * P])
                            lhs_ap = lhsT[:]
                        else:
                            lhs_ap = yT[:, t * P:(t + 1) * P]
                        nc.tensor.matmul(ph[:], lhsT=lhs_ap, rhs=wk[k][:, :HC],
                                         start=(k == 0), stop=(k == kchunks - 1))
                        nc.tensor.matmul(psd[:], lhsT=lhs_ap, rhs=wk[k][:, HC:HC + 2 * H],
                                         start=(k == 0), stop=(k == kchunks - 1))
                    hs = sb3.tile([P, R], F32, tag="hs")
                    if "mmcopy" not in ablate:
                        nc.scalar.copy(hs[:, :HC], ph[:])
                        nc.vector.tensor_copy(hs[:, s_off:s_off + H], psd[:, :H])
                        if R > s_off + H:
                            nc.vector.memset(hs[:, s_off + H:], 0.0)
                        nc.vector.tensor_copy(d_buf[:, t * H:(t + 1) * H], psd[:, H:2 * H])
                    if t == ntiles - 1:
                        # pad nodes: s += -1e30 so padded slots die in the softmax
                        nc.vector.tensor_tensor(out=hs[:, s_off:s_off + H],
                                                in0=hs[:, s_off:s_off + H],
                                                in1=spad_t[:].broadcast_to([P, H]),
                                                op=mybir.AluOpType.add)
                    if "mmstore" not in ablate and "mmcopy" not in ablate:
                        nc.sync.dma_start(ag_in[t * P:(t + 1) * P, :], hs[:])

                # ---- phase 2: AllGather the node table ----
                if "coll" not in ablate:
                    nc.gpsimd.collective_compute(
                        "AllGather", mybir.AluOpType.bypass,
                        replica_groups=[list(range(NCORES))],
                        ins=[ag_in.opt()], outs=[table.opt()],
                    )

                # ---- phase 3: gather + segment softmax + weighted accumulation ----
                oT = sb.tile([P, shard_pad], F32, tag="oT")
                GRP = 4
                for g0 in range(0, ntiles, GRP):
                    gs = min(GRP, ntiles - g0)
                    accg = sb2.tile([P, GRP * HC], F32, tag="accg")
                    deng = sb12.tile([P, GRP * H], F32, tag="deng")
                    for gi in range(gs):
                        t = g0 + gi
                        st = S[t]
                        d_ap = d_buf[:, t * H:(t + 1) * H]
                        acc_slice = accg[:, gi * HC:(gi + 1) * HC]
                        den_slice = deng[:, gi * H:(gi + 1) * H]
                        acc = den = m = None
                        j0 = 0
                        while j0 < st:
                            jc = min(CHUNK, st - j0)
                            last = (j0 + jc >= st)
                            hg = sb3.tile([P, CHUNK * R], F32, tag="hg")
                            for j in range(jc if "gather" not in ablate else 0):
                                nc.gpsimd.indirect_dma_start(
                                    out=hg[:, (j * R):(j * R + R)],
                                    out_offset=None,
                                    in_=table[:],
                                    in_offset=bass.IndirectOffsetOnAxis(
                                        ap=idx_t[:, offs[t] + j0 + j: offs[t] + j0 + j + 1],
                                        axis=0),
                                )
                            if "edge" in ablate:
                                j0 += jc
                                continue
                            hg3 = hg[:].rearrange("p (j r) -> p j r", j=CHUNK)
                            e2 = sb12.tile([P, H * CHUNK], F32, tag="e2")
                            e2v = e2[:, :H * jc].rearrange("p (h j) -> p h j", h=H)
                            nc.vector.tensor_tensor(
                                out=e2v,
                                in0=hg3[:, :jc, s_off:s_off + H].transpose([0, 2, 1]),
                                in1=d_ap.unsqueeze(2).broadcast_to([P, H, jc]),
                                op=mybir.AluOpType.add)
                            nc.vector.scalar_tensor_tensor(
                                out=e2v, in0=e2v, scalar=NEG_SLOPE, in1=e2v,
                                op0=mybir.AluOpType.mult, op1=mybir.AluOpType.max)
                            mc = sb12.tile([P, H], F32, tag="mc")
                            nc.vector.tensor_reduce(out=mc[:], in_=e2v,
                                                    axis=mybir.AxisListType.X, op=mybir.AluOpType.max)
                            if m is not None:
                                mnew = sb12.tile([P, H], F32, tag="mc")
                                nc.vector.tensor_tensor(out=mnew[:], in0=m[:], in1=mc[:], op=mybir.AluOpType.max)
                                so = sb12.tile([P, H], F32, tag="so")
                                nc.vector.tensor_tensor(out=so[:], in0=m[:], in1=mnew[:], op=mybir.AluOpType.subtract)
                                nc.scalar.activation(so[:], so[:], mybir.ActivationFunctionType.Exp)
                                m = mnew
                            else:
                                m = mc
                            pb = sb12.tile([P, H * CHUNK], F32, tag="pb")
                            pbv = pb[:, :H * jc].rearrange("p (h j) -> p h j", h=H)
                            nc.vector.tensor_tensor(out=pbv, in0=e2v,
                                                    in1=m[:].unsqueeze(2).broadcast_to([P, H, jc]),
                                                    op=mybir.AluOpType.subtract)
                            nc.scalar.activation(pb[:, :H * jc], pb[:, :H * jc],
                                                 mybir.ActivationFunctionType.Exp)
                            if last and den is None:
                                ds_out = den_slice
                            else:
                                ds_tile = sb12.tile([P, H], F32, tag="ds")
                                ds_out = ds_tile[:]
                            nc.vector.tensor_reduce(out=ds_out, in_=pbv,
                                                    axis=mybir.AxisListType.X, op=mybir.AluOpType.add)
                            if den is not None:
                                if last:
                                    dn = den_slice
                                else:
                                    dn_tile = sb12.tile([P, H], F32, tag="ds")
                                    dn = dn_tile[:]
                                nc.vector.tensor_tensor(out=dn, in0=den, in1=so[:], op=mybir.AluOpType.mult)
                                nc.vector.tensor_tensor(out=dn, in0=dn, in1=ds_out, op=mybir.AluOpType.add)
                                den = dn
                            else:
                                den = ds_out
                            hgw = sb3.tile([P, CHUNK * R], F32, tag="hg")
                            nc.vector.tensor_tensor(
                                out=hgw[:, :jc * HC].rearrange("p (j h c) -> p j h c", j=jc, h=H),
                                in0=hg3[:, :jc, :HC].rearrange("p j (h c) -> p j h c", h=H),
                                in1=pb[:, :H * jc].rearrange("p (h j) -> p h j", h=H)
                                    .transpose([0, 2, 1]).unsqueeze(3).broadcast_to([P, jc, H, C]),
                                op=mybir.AluOpType.mult)
                            if last and acc is None:
                                red_out = acc_slice
                            else:
                                red_tile = sb6.tile([P, HC], F32, tag="acc")
                                red_out = red_tile[:]
                            nc.vector.tensor_tensor(out=red_out, in0=hgw[:, :HC],
                                                    in1=hgw[:, HC:2 * HC] if jc > 1 else hgw[:, :HC],
                                                    op=mybir.AluOpType.add if jc > 1 else mybir.AluOpType.bypass)
                            for jj in range(2, jc):
                                nc.vector.tensor_tensor(out=red_out, in0=red_out,
                                                        in1=hgw[:, jj * HC:(jj + 1) * HC],
                                                        op=mybir.AluOpType.add)
                            if acc is not None:
                                if last:
                                    an = acc_slice
                                else:
                                    an_tile = sb6.tile([P, HC], F32, tag="acc")
                                    an = an_tile[:]
                                nc.vector.tensor_tensor(
                                    out=an.rearrange("p (h c) -> p h c", h=H),
                                    in0=acc.rearrange("p (h c) -> p h c", h=H),
                                    in1=so[:].unsqueeze(2).broadcast_to([P, H, C]),
                                    op=mybir.AluOpType.mult)
                                nc.vector.tensor_tensor(out=an, in0=an, in1=red_out, op=mybir.AluOpType.add)
                                acc = an
                            else:
                                acc = red_out
                            j0 += jc

                    if "edge" in ablate or "epi" in ablate:
                        continue
                    # group-wide: o = sum_h acc / ((den + 1e-16) * H)
                    rcp = sb12.tile([P, GRP * H], F32, tag="rcpg")
                    nc.vector.tensor_scalar_add(rcp[:, :gs * H], deng[:, :gs * H], 1e-16)
                    nc.vector.reciprocal(rcp[:, :gs * H], rcp[:, :gs * H])
                    if H > 1:
                        nc.vector.tensor_scalar_mul(rcp[:, :gs * H], rcp[:, :gs * H], 1.0 / H)
                    nc.vector.tensor_tensor(
                        out=accg[:, :gs * HC].rearrange("p (g h c) -> p g h c", g=gs, h=H),
                        in0=accg[:, :gs * HC].rearrange("p (g h c) -> p g h c", g=gs, h=H),
                        in1=rcp[:, :gs * H].rearrange("p (g h) -> p g h", g=gs).unsqueeze(3)
                            .broadcast_to([P, gs, H, C]),
                        op=mybir.AluOpType.mult)
                    if H > 1:
                        og = sb6.tile([P, GRP * C], F32, tag="og")
                        a4 = accg[:, :gs * HC].rearrange("p (g h c) -> p g h c", g=gs, h=H)
                        nc.vector.tensor_tensor(out=og[:, :gs * C].rearrange("p (g c) -> p g c", g=gs),
                                                in0=a4[:, :, 0, :], in1=a4[:, :, 1, :],
                                                op=mybir.AluOpType.add)
                        for hh in range(2, H):
                            nc.vector.tensor_tensor(out=og[:, :gs * C].rearrange("p (g c) -> p g c", g=gs),
                                                    in0=og[:, :gs * C].rearrange("p (g c) -> p g c", g=gs),
                                                    in1=a4[:, :, hh, :],
                                                    op=mybir.AluOpType.add)
                        osrc = og
                    else:
                        osrc = accg
                    for gi in range(gs):
                        t = g0 + gi
                        ptr = psum4.tile([P, P], F32, tag="tr", space="PSUM")
                        nc.tensor.transpose(out=ptr[:], in_=osrc[:, gi * C:(gi + 1) * C], identity=ident[:])
                        nc.vector.tensor_copy(oT[:, t * P:(t + 1) * P], ptr[:])

                # ---- phase 4: batchnorm (+relu) ----
                nsum = sb4.tile([P, 1], F32, tag="nsum")
                nsq = sb4.tile([P, 1], F32, tag="nsq")
                nc.vector.tensor_reduce(out=nsum[:], in_=oT[:], axis=mybir.AxisListType.X,
                                        op=mybir.AluOpType.add)
                yTn = sb.tile([P, shard_pad], F32, tag="yT{}".format(l % 2))
                nc.scalar.activation(yTn[:], oT[:], mybir.ActivationFunctionType.Square,
                                     accum_out=nsq[:])
                ar_in = dram.tile([P, 2], F32, tag=f"arin{l}")
                ar_out = dram.tile([P, 2], F32, tag=f"arout{l}")
                st2 = sb4.tile([P, 2], F32, tag="st2")
                nc.vector.tensor_copy(st2[:, 0:1], nsum[:])
                nc.vector.tensor_copy(st2[:, 1:2], nsq[:])
                nc.gpsimd.dma_start(ar_in[:], st2[:])
                if "coll" not in ablate:
                    nc.gpsimd.collective_compute(
                        "AllReduce", mybir.AluOpType.add,
                        replica_groups=[list(range(NCORES))],
                        ins=[ar_in.opt()], outs=[ar_out.opt()],
                    )
                stg = sb4.tile([P, 2], F32, tag="stg")
                nc.sync.dma_start(stg[:], ar_out[:])
                ntotal = float(NCORES * g["shard"])
                mu = sb4.tile([P, 1], F32, tag="mu")
                nc.vector.tensor_scalar_mul(mu[:], stg[:, 0:1], 1.0 / ntotal)
                var = sb4.tile([P, 1], F32, tag="var")
                nc.vector.tensor_scalar_mul(var[:], stg[:, 1:2], 1.0 / ntotal)
                musq = sb4.tile([P, 1], F32, tag="musq")
                nc.vector.tensor_tensor(out=musq[:], in0=mu[:], in1=mu[:], op=mybir.AluOpType.mult)
                nc.vector.tensor_tensor(out=var[:], in0=var[:], in1=musq[:], op=mybir.AluOpType.subtract)
                rstd = sb4.tile([P, 1], F32, tag="rstd")
                nc.vector.tensor_scalar_add(var[:], var[:], EPS_BN)
                nc.scalar.activation(rstd[:], var[:], mybir.ActivationFunctionType.Sqrt)
                nc.vector.reciprocal(rstd[:], rstd[:])
                scale = sb4.tile([P, 1], F32, tag="scale")
                nc.vector.tensor_tensor(out=scale[:], in0=gb_t[:, 2 * l:2 * l + 1], in1=rstd[:],
                                        op=mybir.AluOpType.mult)
                shift = sb4.tile([P, 1], F32, tag="shift")
                nc.vector.tensor_tensor(out=shift[:], in0=mu[:], in1=scale[:], op=mybir.AluOpType.mult)
                nc.vector.tensor_tensor(out=shift[:], in0=gb_t[:, 2 * l + 1:2 * l + 2], in1=shift[:],
                                        op=mybir.AluOpType.subtract)
                func = (mybir.ActivationFunctionType.Relu if l < n_l - 1
                        else mybir.ActivationFunctionType.Identity)
                nc.scalar.activation(yTn[:], oT[:], func, bias=shift[:], scale=scale[:])
                npad = shard_pad - g["shard"]
                if npad > 0 and l < n_l - 1:
                    nc.vector.memset(yTn[:, g["shard"]:], 0.0)
                yT = yTn

            nc.sync.dma_start(out_t[:], yT[:])

    nc.compile()
    return nc


# ----------------------------------------------------------------------------
# entry point
# ----------------------------------------------------------------------------

def build_for_inputs(x, edge_index, params_list, ablate=(), nlayers=3):
    """Build (nc, in_maps) without running. params_list = [(W, a_src, a_dst, gamma, beta), ...]"""
    x = np.asarray(x, np.float32)
    N, in_dim = x.shape
    g = _prep(np.asarray(edge_index), N)
    params = params_list[:nlayers]
    layers = []
    for (W, asr, ads, gmm, bet) in params:
        H, C = asr.shape
        HC = H * C
        R = ((HC + H) * 4 + 31) // 32 * 8
        layers.append({"H": H, "C": C, "R": R, "hs_off": HC})
    nc = _build_program(g, layers, in_dim, ablate=ablate)
    wexts = []
    for (W, asr, ads, gmm, bet), L in zip(params, layers):
        H, C = L["H"], L["C"]
        w_s = np.einsum("khc,hc->kh", W.reshape(W.shape[0], H, C), asr)
        w_d = np.einsum("khc,hc->kh", W.reshape(W.shape[0], H, C), ads)
        wexts.append(np.concatenate([W, w_s, w_d], axis=1).astype(np.float32))
    gbm = np.zeros((P, 2 * len(layers)), np.float32)
    for l, (W, asr, ads, gmm, bet) in enumerate(params):
        gbm[:len(gmm), 2 * l] = gmm
        gbm[:len(bet), 2 * l + 1] = bet
    shard, shard_pad = g["shard"], g["shard_pad"]
    in_maps = []
    for c in range(NCORES):
        nodes = g["out_nodes"][c]
        xT_c = np.zeros((in_dim, shard_pad), np.float32)
        xT_c[:, :shard] = x[nodes].T
        spad = np.zeros((P, 1), np.float32)
        lastbase = (g["ntiles"] - 1) * P
        for p in range(P):
            if lastbase + p >= shard:
                spad[p, 0] = -1e30
        m = {"xT": xT_c, "idx": np.ascontiguousarray(g["idx"][c]), "gb": gbm, "spad": spad}
        for l, w in enumerate(wexts):
            m[f"wext{l}"] = w
        in_maps.append(m)
    return nc, in_maps, g, layers


def kernel(x, edge_index,
           W0, a_src0, a_dst0, b0, gamma0, beta0,
           W1, a_src1, a_dst1, b1, gamma1, beta1,
           W2, a_src2, a_dst2, b2, gamma2, beta2, _profile=None, _nlayers=3):
    x = np.asarray(x, np.float32)
    N, in_dim = x.shape
    g = _prep(np.asarray(edge_index), N)

    params = [(np.asarray(W0, np.float32), np.asarray(a_src0, np.float32), np.asarray(a_dst0, np.float32),
               np.asarray(gamma0, np.float32), np.asarray(beta0, np.float32)),
              (np.asarray(W1, np.float32), np.asarray(a_src1, np.float32), np.asarray(a_dst1, np.float32),
               np.asarray(gamma1, np.float32), np.asarray(beta1, np.float32)),
              (np.asarray(W2, np.float32), np.asarray(a_src2, np.float32), np.asarray(a_dst2, np.float32),
               np.asarray(gamma2, np.float32), np.asarray(beta2, np.float32))]

    params = params[:_nlayers]
    layers = []
    for (W, asr, ads, gmm, bet) in params:
        H, C = asr.shape
        HC = H * C
        R = ((HC + H) * 4 + 31) // 32 * 8  # row f32 elems, 32B-aligned
        layers.append({"H": H, "C": C, "R": R, "hs_off": HC})

    nc = _build_program(g, layers, in_dim)

    # per-layer extended weights [K, H*C + 2H] = [W | W@a_src^T per head | W@a_dst^T]
    wexts = []
    for (W, asr, ads, gmm, bet), L in zip(params, layers):
        H, C = L["H"], L["C"]
        w_s = np.einsum("khc,hc->kh", W.reshape(W.shape[0], H, C), asr)
        w_d = np.einsum("khc,hc->kh", W.reshape(W.shape[0], H, C), ads)
        wexts.append(np.concatenate([W, w_s, w_d], axis=1).astype(np.float32))

    gb = np.zeros((P, 2 * len(layers)), np.float32)
    for l, (W, asr, ads, gmm, bet) in enumerate(params):
        gb[:len(gmm), 2 * l] = gmm
        gb[:len(bet), 2 * l + 1] = bet

    shard, shard_pad = g["shard"], g["shard_pad"]
    in_maps = []
    for c in range(NCORES):
        nodes = g["out_nodes"][c]
        xT_c = np.zeros((in_dim, shard_pad), np.float32)
        xT_c[:, :shard] = x[nodes].T
        spad = np.zeros((P, 1), np.float32)
        lastbase = (g["ntiles"] - 1) * P
        for p in range(P):
            if lastbase + p >= shard:
                spad[p, 0] = -1e30
        m = {"xT": xT_c, "idx": np.ascontiguousarray(g["idx"][c]), "gb": gb, "spad": spad}
        for l, w in enumerate(wexts):
            m[f"wext{l}"] = w
        in_maps.append(m)

    if _profile is not None:
        _profile["nc"] = nc
        _profile["in_maps"] = in_maps
    res = bass_utils.run_bass_kernel_spmd(nc, in_maps, core_ids=list(range(NCORES)))

    C_out = layers[-1]["C"]
    out = np.empty((N, C_out), np.float32)
    for c in range(NCORES):
        yT = res.results[c]["out"]           # [P(feat), shard_pad]
        out[g["out_nodes"][c]] = yT[:C_out, :shard].T
    if _profile is not None:
        _profile["results"] = res
    return out



# revision 21
# speedup vs baseline: 2.9652x; 2.9652x over previous
"""GAT node encoder (3 GATConv+BN layers) on 8 trn2 NeuronCores.

Sharding: nodes partitioned across cores (dst-sharded message passing).
Per layer, per core:
  1. local matmul of this core's node shard: [h | s | d] = y @ [W | W@a_src | W@a_dst]
     (bf16 table rows [h | s | pad] written to DRAM)
  2. AllGather of the bf16 node table (node-major rows) across cores
  3. per dst-tile (128 nodes, degree-sorted ELL layout): bulk row gathers of
     h[src] via dma_gather (int16 indices -> two overlapped address windows
     A=[0,32768) and B=[17408,50176)), exact per-tile segment softmax,
     weighted accumulation, head mean
  4. BatchNorm: feature-major stats via free-axis reduction + AllReduce of
     per-feature sums, fused scale/shift(+ReLU) activation.

The per-feature bias b is dropped: BN(o + b) == BN(o) exactly.
"""
import os
import sys

sys.path.insert(0, "/opt/trn_rl_repo")

import numpy as np
import ml_dtypes

import concourse.bass as bass
import concourse.bacc as bacc
import concourse.tile as tile
from concourse import mybir
from concourse import bass_utils
from concourse.masks import make_identity

NCORES = 8
P = 128
NEG_SLOPE = 0.2
EPS_BN = 1e-5
GCAP = 24          # max ELL slots (A+B) packed per gather group
MAXCOL = 8         # max 128-row columns per dma_gather call (desc ring limit)
CH = 14            # mult/reduce chunk width (slots)
LOWN = 32768       # window A rows [0, 32768)
OVER = 17408       # window B base: rows [17408, 50176)
CORE_ORDER = [3, 4, 2, 5, 1, 6, 0, 7]  # out-deg-desc rank -> core (overlap rows get high fanout)

F32 = mybir.dt.float32
BF16 = mybir.dt.bfloat16
I16 = mybir.dt.int16

BF = ml_dtypes.bfloat16


# ----------------------------------------------------------------------------
# host-side graph preprocessing
# ----------------------------------------------------------------------------

def _prep(edge_index, N):
    src = np.asarray(edge_index[0], dtype=np.int64)
    dst = np.asarray(edge_index[1], dtype=np.int64)
    loops = np.arange(N, dtype=np.int64)
    src = np.concatenate([src, loops])
    dst = np.concatenate([dst, loops])

    shard = N // NCORES
    ntiles = (shard + P) // P                # always >= 1 pad row per shard
    shard_pad = ntiles * P
    nrows = NCORES * shard_pad

    deg_in = np.bincount(dst, minlength=N)
    deg_out = np.bincount(src, minlength=N)

    # node -> table row: cores get contiguous out-degree-desc slices (so the
    # window-overlap rows [OVER, LOWN) hold max-fanout sources); within a core
    # rows are in-degree-desc (tight ELL tiles).
    order = np.argsort(-deg_out, kind="stable")
    node_row = np.empty(N, np.int64)
    core_nodes = []
    for c in range(NCORES):
        core_nodes.append(None)
    for rank_c in range(NCORES):
        c = CORE_ORDER[rank_c]
        nodes = order[rank_c * shard:(rank_c + 1) * shard]
        o2 = np.argsort(-deg_in[nodes], kind="stable")
        nodes = nodes[o2]
        node_row[nodes] = c * shard_pad + np.arange(shard)
        core_nodes[c] = nodes

    src_row = node_row[src]
    dst_row = node_row[dst]
    c_of = dst_row // shard_pad
    r_of = dst_row % shard_pad

    over = max(0, nrows - LOWN)   # window B base (17408 full-size, 0 if tiny)

    # per-(core,row) bucketed src rows, A-forced first then flex then B-forced
    forcedB = src_row >= LOWN
    flex = (src_row >= over) & (~forcedB)
    keyclass = np.where(forcedB, 2, np.where(flex, 1, 0))
    o = np.lexsort((keyclass, r_of, c_of))
    src_s, c_s, r_s, k_s = src_row[o], c_of[o], r_of[o], keyclass[o]

    nT = np.zeros((NCORES, shard_pad), np.int64)
    np.add.at(nT, (c_of, r_of), 1)
    nA = np.zeros((NCORES, shard_pad), np.int64)
    np.add.at(nA, (c_of[keyclass == 0], r_of[keyclass == 0]), 1)
    nB = np.zeros((NCORES, shard_pad), np.int64)
    np.add.at(nB, (c_of[keyclass == 2], r_of[keyclass == 2]), 1)

    FA = nA.reshape(NCORES, ntiles, P).max(axis=2).max(axis=0)
    FB = nB.reshape(NCORES, ntiles, P).max(axis=2).max(axis=0)
    S = nT.reshape(NCORES, ntiles, P).max(axis=2).max(axis=0)
    S_A = np.maximum(FA, 1)
    S_B = np.maximum(FB, np.maximum(S, FA + FB) - S_A)
    S_B = np.maximum(S_B, 1)

    # per-(core,row) final A-count: a = max(forcedA, tot - S_B[tile])
    tilev = np.arange(shard_pad) // P
    a_fin = np.maximum(nA, nT - S_B[None, tilev])

    # slot tables: for each (c, r): first a_fin A-slots then the rest B-slots
    # (bucketed edge list is ordered forcedA, flex, forcedB per row)
    run_starts = np.zeros((NCORES, shard_pad), np.int64)
    flat_counts = nT.reshape(-1)
    run_starts.reshape(-1)[:] = np.concatenate([[0], np.cumsum(flat_counts)[:-1]])

    # group packing
    groups = []
    t0 = 0
    while t0 < ntiles:
        t1 = t0 + 1
        tot = S_A[t0] + S_B[t0]
        while t1 < ntiles and tot + S_A[t1] + S_B[t1] <= GCAP:
            tot += S_A[t1] + S_B[t1]
            t1 += 1
        groups.append((t0, t1))
        t0 = t1

    PAD_A = shard                      # core-0 pad row (< LOWN), s = -1e30
    PAD_B = (NCORES - 1) * shard_pad + shard   # core-7 pad row (>= over)

    # build per-core idx16 lists + per-group metadata
    gmeta = []                 # per group: dict with call/col info
    idx_cols = []              # per core: list of columns (each 128 int16)
    percore_cols = [[] for _ in range(NCORES)]
    o16 = 0
    for (t0, t1) in groups:
        ncolsA = int(S_A[t0:t1].sum())
        ncolsB = int(S_B[t0:t1].sum())
        meta = {
            "t0": t0, "t1": t1,
            "ncolsA": ncolsA, "ncolsB": ncolsB,
            "offA": [], "offB": [], "calls": [],
        }
        off = 0
        for t in range(t0, t1):
            meta["offA"].append(off)
            off += int(S_A[t])
        for t in range(t0, t1):
            meta["offB"].append(off)
            off += int(S_B[t])
        # split each window's columns into <=MAXCOL-column gather calls
        # (SWDGE descriptor ring limit: 1024 descriptors per instruction)
        for win, ncols, base in (("A", ncolsA, 0), ("B", ncolsB, ncolsA)):
            c0 = 0
            while c0 < ncols:
                cw = min(MAXCOL, ncols - c0)
                meta["calls"].append((win, o16, cw, base + c0))
                o16 += cw * 8
                c0 += cw
        gmeta.append(meta)

        for c in range(NCORES):
            colsA, colsB = [], []
            for t in range(t0, t1):
                sa, sb = int(S_A[t]), int(S_B[t])
                colA = np.full((sa, P), PAD_A, np.int64)
                colB = np.full((sb, P), PAD_B - over, np.int64)
                for p in range(P):
                    r = t * P + p
                    cnt = int(nT[c, r])
                    if cnt == 0:
                        continue
                    st = run_starts[c, r]
                    rows = src_s[st:st + cnt]
                    na = int(a_fin[c, r])
                    colA[:na, p] = rows[:na]
                    colB[:cnt - na, p] = rows[na:] - over
                colsA.append(colA)
                colsB.append(colB)
            percore_cols[c].append((np.concatenate(colsA, 0) if colsA else
                                    np.zeros((0, P), np.int64),
                                    np.concatenate(colsB, 0) if colsB else
                                    np.zeros((0, P), np.int64)))

    W16 = o16
    idx16 = np.zeros((NCORES, 128, W16), np.int16)
    for c in range(NCORES):
        pos = 0
        for gi, (ca, cb) in enumerate(percore_cols[c]):
            for arr in (ca, cb):
                ncols = arr.shape[0]
                if ncols == 0:
                    continue
                flat = arr.reshape(-1)            # position i = col*128 + p
                wrapped = flat.reshape(-1, 16).T  # [16, ncols*8]
                idx16[c, :, pos:pos + ncols * 8] = np.tile(wrapped, (8, 1))
                pos += ncols * 8
        assert pos == W16

    out_of_core = [core_nodes[c] for c in range(NCORES)]
    return {
        "shard": shard, "shard_pad": shard_pad, "ntiles": ntiles,
        "S_A": S_A.astype(int).tolist(), "S_B": S_B.astype(int).tolist(),
        "groups": groups, "gmeta": gmeta, "W16": W16, "idx16": idx16,
        "node_row": node_row, "out_nodes": out_of_core, "over": over,
    }


# ----------------------------------------------------------------------------
# device program
# ----------------------------------------------------------------------------

def _build_program(g, layers, in_dim, ablate=()):
    """layers: list of dicts {H, C, R, hs_off} per layer.
    R = table row bf16 elems (h | s | pad), 128-elem aligned. hs_off = H*C.
    """
    ablate = set(ablate)
    shard_pad, ntiles = g["shard_pad"], g["ntiles"]
    S_A, S_B = g["S_A"], g["S_B"]
    gmeta, W16 = g["gmeta"], g["W16"]
    nrows = NCORES * shard_pad
    n_l = len(layers)

    nc = bacc.Bacc("TRN2", target_bir_lowering=False, debug=False, num_devices=NCORES)

    xT = nc.dram_tensor("xT", [in_dim, shard_pad], BF16, kind="ExternalInput").ap()
    idx_in = nc.dram_tensor("idx", [128, W16], I16, kind="ExternalInput").ap()
    wexts = [nc.dram_tensor(f"wext{l}", [(in_dim if l == 0 else layers[l - 1]["C"]),
                                         layers[l]["H"] * layers[l]["C"] + 2 * layers[l]["H"]],
                            BF16, kind="ExternalInput").ap() for l in range(n_l)]
    gb = nc.dram_tensor("gb", [P, 2 * n_l], F32, kind="ExternalInput").ap()
    spad_in = nc.dram_tensor("spad", [P, 1], BF16, kind="ExternalInput").ap()
    out_t = nc.dram_tensor("out", [P, shard_pad], F32, kind="ExternalOutput").ap()

    with tile.TileContext(nc) as tc:
        import contextlib
        with contextlib.ExitStack() as ctx:
            dram = ctx.enter_context(tc.tile_pool(name="dram", bufs=1, space="DRAM"))
            psum = ctx.enter_context(tc.tile_pool(name="psum", bufs=2, space="PSUM"))
            psum4 = ctx.enter_context(tc.tile_pool(name="psum4", bufs=4, space="PSUM"))
            sb = ctx.enter_context(tc.tile_pool(name="sb", bufs=1))
            sb2 = ctx.enter_context(tc.tile_pool(name="sb2", bufs=2))
            sb3 = ctx.enter_context(tc.tile_pool(name="sb3", bufs=3))
            sb4 = ctx.enter_context(tc.tile_pool(name="sb4", bufs=4))
            sbhg = ctx.enter_context(tc.tile_pool(name="sbhg", bufs=2))
            sbhw = ctx.enter_context(tc.tile_pool(name="sbhw", bufs=2))
            sbe = ctx.enter_context(tc.tile_pool(name="sbe", bufs=4))
            sbacc = ctx.enter_context(tc.tile_pool(name="sbacc", bufs=2))
            sbt = ctx.enter_context(tc.tile_pool(name="sbt", bufs=2))

            ident = sb.tile([P, P], F32, tag="ident")
            make_identity(nc, ident[:])
            idx_t = sb.tile([128, W16], I16, tag="idx")
            nc.sync.dma_start(idx_t[:], idx_in[:])
            gb_t = sb.tile([P, 2 * n_l], F32, tag="gb")
            nc.sync.dma_start(gb_t[:], gb[:])
            spad_t = sb.tile([P, 1], BF16, tag="spad")
            nc.sync.dma_start(spad_t[:], spad_in[:])

            yT = None  # [P(feat), shard_pad] SBUF bf16, input to next layer
            for l, L in enumerate(layers):
                H, C, R, s_off = L["H"], L["C"], L["R"], L["hs_off"]
                HC = H * C
                K = in_dim if l == 0 else layers[l - 1]["C"]
                kchunks = K // P

                ag_in = dram.tile([shard_pad, R], BF16, tag=f"agin{l}")
                table = dram.tile([nrows, R], BF16, tag=f"table{l}",
                                  addr_space=("Shared" if os.environ.get("K_SHARED") else "Local"))

                wk = []
                for k in range(kchunks):
                    w = sb2.tile([P, HC + 2 * H], BF16, tag="wext")
                    nc.sync.dma_start(w[:], wexts[l][k * P:(k + 1) * P, :])
                    wk.append(w)

                d_buf = sb.tile([P, ntiles * H], F32, tag=f"dbuf{l % 2}")

                # ---- phase 1: local shard matmul -> hs rows + d ----
                for t in range(ntiles):
                    ph = psum.tile([P, HC], F32, tag="mmh", space="PSUM")
                    psd = psum.tile([P, 2 * H], F32, tag="mmsd", space="PSUM")
                    for k in range(kchunks if "mm" not in ablate else 0):
                        if yT is None:
                            lhsT = sb4.tile([P, P], BF16, tag="xt")
                            nc.sync.dma_start(lhsT[:], xT[k * P:(k + 1) * P, t * P:(t + 1) * P])
                            lhs_ap = lhsT[:]
                        else:
                            lhs_ap = yT[:, t * P:(t + 1) * P]
                        nc.tensor.matmul(ph[:], lhsT=lhs_ap, rhs=wk[k][:, :HC],
                                         start=(k == 0), stop=(k == kchunks - 1))
                        nc.tensor.matmul(psd[:], lhsT=lhs_ap, rhs=wk[k][:, HC:HC + 2 * H],
                                         start=(k == 0), stop=(k == kchunks - 1))
                    hs = sb3.tile([P, R], BF16, tag="hs")
                    if "mmcopy" not in ablate:
                        nc.scalar.copy(hs[:, :HC], ph[:])
                        nc.vector.tensor_copy(hs[:, s_off:s_off + H], psd[:, :H])
                        nc.vector.memset(hs[:, s_off + H:], 0.0)
                        nc.vector.tensor_copy(d_buf[:, t * H:(t + 1) * H], psd[:, H:2 * H])
                    if t == ntiles - 1:
                        # pad nodes: s += -1e30 so padded slots die in the softmax
                        nc.vector.tensor_tensor(out=hs[:, s_off:s_off + H],
                                                in0=hs[:, s_off:s_off + H],
                                                in1=spad_t[:].broadcast_to([P, H]),
                                                op=mybir.AluOpType.add)
                    if "mmstore" not in ablate and "mmcopy" not in ablate:
                        nc.sync.dma_start(ag_in[t * P:(t + 1) * P, :], hs[:])

                # ---- phase 2: AllGather the node table ----
                if "coll" not in ablate:
                    nc.gpsimd.collective_compute(
                        "AllGather", mybir.AluOpType.bypass,
                        replica_groups=[list(range(NCORES))],
                        ins=[ag_in.opt()], outs=[table.opt()],
                    )

                # ---- phase 3: gather + segment softmax + weighted accumulation ----
                over = g["over"]
                winA = min(LOWN, nrows)
                oT = sb.tile([P, shard_pad], F32, tag="oT")
                if "edge" in ablate:
                    nc.vector.memset(oT[:], 0.0)
                for gi, meta in enumerate(gmeta):
                    t0, t1 = meta["t0"], meta["t1"]
                    gtiles = t1 - t0
                    ncolsA, ncolsB = meta["ncolsA"], meta["ncolsB"]
                    gcols = ncolsA + ncolsB
                    hg = sbhg.tile([P, gcols * R], BF16, tag="hg")
                    if "gather" not in ablate:
                        for (win, o16, cw, hc0) in meta["calls"]:
                            nc.gpsimd.dma_gather(
                                out_ap=hg[:, hc0 * R:(hc0 + cw) * R]
                                    .rearrange("p (b r) -> p b r", r=R),
                                in_ap=(table[:winA, :] if win == "A"
                                       else table[over:, :]),
                                idxs_ap=idx_t[:, o16:o16 + cw * 8],
                                num_idxs=cw * 128, num_idxs_reg=cw * 128,
                                elem_size=R)
                    if "edge" in ablate:
                        continue
                    accg = sbacc.tile([P, gtiles * HC], F32, tag="accg")
                    deng = sbe.tile([P, gtiles * H], F32, tag="deng")
                    for ti in range(gtiles):
                        t = t0 + ti
                        sa, sbw = S_A[t], S_B[t]
                        st = sa + sbw
                        oA, oB = meta["offA"][ti], meta["offB"][ti]
                        d_ap = d_buf[:, t * H:(t + 1) * H]
                        acc_slice = accg[:, ti * HC:(ti + 1) * HC]
                        den_slice = deng[:, ti * H:(ti + 1) * H]

                        # scores e = lrelu(s_src + d_dst), exact per-tile max
                        e2 = sbe.tile([P, H * st], F32, tag="e2")
                        e2v = e2[:].rearrange("p (h j) -> p h j", h=H)
                        for (oo, w0, jo) in ((oA, sa, 0), (oB, sbw, sa)):
                            hg3 = hg[:, oo * R:(oo + w0) * R].rearrange(
                                "p (j r) -> p j r", j=w0)
                            nc.vector.tensor_tensor(
                                out=e2v[:, :, jo:jo + w0],
                                in0=hg3[:, :, s_off:s_off + H].transpose([0, 2, 1]),
                                in1=d_ap.unsqueeze(2).broadcast_to([P, H, w0]),
                                op=mybir.AluOpType.add)
                        nc.vector.scalar_tensor_tensor(
                            out=e2[:], in0=e2[:], scalar=NEG_SLOPE, in1=e2[:],
                            op0=mybir.AluOpType.mult, op1=mybir.AluOpType.max)
                        m = sbe.tile([P, H], F32, tag="mx")
                        nc.vector.tensor_reduce(out=m[:], in_=e2v,
                                                axis=mybir.AxisListType.X,
                                                op=mybir.AluOpType.max)
                        nc.vector.tensor_tensor(
                            out=e2v, in0=e2v,
                            in1=m[:].unsqueeze(2).broadcast_to([P, H, st]),
                            op=mybir.AluOpType.subtract)
                        pb = sbe.tile([P, H * st], BF16, tag="pb")
                        nc.scalar.activation(pb[:], e2[:],
                                             mybir.ActivationFunctionType.Exp)
                        pb3 = pb[:].rearrange("p (h j) -> p h j", h=H)
                        nc.vector.tensor_reduce(out=den_slice, in_=pb3,
                                                axis=mybir.AxisListType.X,
                                                op=mybir.AluOpType.add)

                        # weighted accumulation: acc[p,hc] = sum_j pb * h
                        first = True
                        for (oo, w0, jo) in ((oA, sa, 0), (oB, sbw, sa)):
                            j0 = 0
                            while j0 < w0:
                                jc = min(CH, w0 - j0)
                                hgw = sbhw.tile([P, CH * HC], BF16, tag="hgw")
                                hgw4 = hgw[:, :jc * HC].rearrange(
                                    "p (h c j) -> p h c j", h=H, c=C)
                                hg4 = hg[:, (oo + j0) * R:(oo + j0 + jc) * R] \
                                    .rearrange("p (j r) -> p j r", j=jc)[:, :, :HC] \
                                    .rearrange("p j (h c) -> p h c j", h=H)
                                nc.vector.tensor_tensor(
                                    out=hgw4, in0=hg4,
                                    in1=pb3[:, :, jo + j0:jo + j0 + jc]
                                        .unsqueeze(2).broadcast_to([P, H, C, jc]),
                                    op=mybir.AluOpType.mult)
                                if first:
                                    red_out = acc_slice
                                else:
                                    tmp = sbt.tile([P, HC], F32, tag="tmp")
                                    red_out = tmp[:]
                                nc.vector.tensor_reduce(
                                    out=red_out,
                                    in_=hgw[:, :jc * HC].rearrange(
                                        "p (hc j) -> p hc j", j=jc),
                                    axis=mybir.AxisListType.X,
                                    op=mybir.AluOpType.add)
                                if not first:
                                    nc.vector.tensor_tensor(
                                        out=acc_slice, in0=acc_slice, in1=red_out,
                                        op=mybir.AluOpType.add)
                                first = False
                                j0 += jc

                    if "epi" in ablate:
                        continue
                    # group-wide: o = sum_h acc / ((den + 1e-16) * H)
                    rcp = sbe.tile([P, gtiles * H], F32, tag="rcpg")
                    nc.vector.tensor_scalar_add(rcp[:, :gtiles * H], deng[:, :gtiles * H], 1e-16)
                    nc.vector.reciprocal(rcp[:, :gtiles * H], rcp[:, :gtiles * H])
                    if H > 1:
                        nc.vector.tensor_scalar_mul(rcp[:, :gtiles * H], rcp[:, :gtiles * H], 1.0 / H)
                    nc.vector.tensor_tensor(
                        out=accg[:, :gtiles * HC].rearrange("p (g h c) -> p g h c", g=gtiles, h=H),
                        in0=accg[:, :gtiles * HC].rearrange("p (g h c) -> p g h c", g=gtiles, h=H),
                        in1=rcp[:, :gtiles * H].rearrange("p (g h) -> p g h", g=gtiles).unsqueeze(3)
                            .broadcast_to([P, gtiles, H, C]),
                        op=mybir.AluOpType.mult)
                    if H > 1:
                        og = sbe.tile([P, gtiles * C], F32, tag="og")
                        a4 = accg[:, :gtiles * HC].rearrange("p (g h c) -> p g h c", g=gtiles, h=H)
                        nc.vector.tensor_tensor(out=og[:, :gtiles * C].rearrange("p (g c) -> p g c", g=gtiles),
                                                in0=a4[:, :, 0, :], in1=a4[:, :, 1, :],
                                                op=mybir.AluOpType.add)
                        for hh in range(2, H):
                            nc.vector.tensor_tensor(out=og[:, :gtiles * C].rearrange("p (g c) -> p g c", g=gtiles),
                                                    in0=og[:, :gtiles * C].rearrange("p (g c) -> p g c", g=gtiles),
                                                    in1=a4[:, :, hh, :],
                                                    op=mybir.AluOpType.add)
                        osrc = og
                    else:
                        osrc = accg
                    for ti in range(gtiles):
                        t = t0 + ti
                        ptr = psum4.tile([P, P], F32, tag="tr", space="PSUM")
                        nc.tensor.transpose(out=ptr[:], in_=osrc[:, ti * C:(ti + 1) * C], identity=ident[:])
                        nc.vector.tensor_copy(oT[:, t * P:(t + 1) * P], ptr[:])

                # ---- phase 4: batchnorm (+relu) ----
                nsum = sb4.tile([P, 1], F32, tag="nsum")
                nsq = sb4.tile([P, 1], F32, tag="nsq")
                nc.vector.tensor_reduce(out=nsum[:], in_=oT[:], axis=mybir.AxisListType.X,
                                        op=mybir.AluOpType.add)
                yF = sb.tile([P, shard_pad], F32, tag="yF")
                nc.scalar.activation(yF[:], oT[:], mybir.ActivationFunctionType.Square,
                                     accum_out=nsq[:])
                ar_in = dram.tile([P, 2], F32, tag=f"arin{l}")
                ar_out = dram.tile([P, 2], F32, tag=f"arout{l}")
                st2 = sb4.tile([P, 2], F32, tag="st2")
                nc.vector.tensor_copy(st2[:, 0:1], nsum[:])
                nc.vector.tensor_copy(st2[:, 1:2], nsq[:])
                nc.gpsimd.dma_start(ar_in[:], st2[:])
                if "coll" not in ablate:
                    nc.gpsimd.collective_compute(
                        "AllReduce", mybir.AluOpType.add,
                        replica_groups=[list(range(NCORES))],
                        ins=[ar_in.opt()], outs=[ar_out.opt()],
                    )
                stg = sb4.tile([P, 2], F32, tag="stg")
                nc.sync.dma_start(stg[:], ar_out[:])
                ntotal = float(NCORES * g["shard"])
                mu = sb4.tile([P, 1], F32, tag="mu")
                nc.vector.tensor_scalar_mul(mu[:], stg[:, 0:1], 1.0 / ntotal)
                var = sb4.tile([P, 1], F32, tag="var")
                nc.vector.tensor_scalar_mul(var[:], stg[:, 1:2], 1.0 / ntotal)
                musq = sb4.tile([P, 1], F32, tag="musq")
                nc.vector.tensor_tensor(out=musq[:], in0=mu[:], in1=mu[:], op=mybir.AluOpType.mult)
                nc.vector.tensor_tensor(out=var[:], in0=var[:], in1=musq[:], op=mybir.AluOpType.subtract)
                rstd = sb4.tile([P, 1], F32, tag="rstd")
                nc.vector.tensor_scalar_add(var[:], var[:], EPS_BN)
                nc.scalar.activation(rstd[:], var[:], mybir.ActivationFunctionType.Sqrt)
                nc.vector.reciprocal(rstd[:], rstd[:])
                scale = sb4.tile([P, 1], F32, tag="scale")
                nc.vector.tensor_tensor(out=scale[:], in0=gb_t[:, 2 * l:2 * l + 1], in1=rstd[:],
                                        op=mybir.AluOpType.mult)
                shift = sb4.tile([P, 1], F32, tag="shift")
                nc.vector.tensor_tensor(out=shift[:], in0=mu[:], in1=scale[:], op=mybir.AluOpType.mult)
                nc.vector.tensor_tensor(out=shift[:], in0=gb_t[:, 2 * l + 1:2 * l + 2], in1=shift[:],
                                        op=mybir.AluOpType.subtract)
                func = (mybir.ActivationFunctionType.Relu if l < n_l - 1
                        else mybir.ActivationFunctionType.Identity)
                if l < n_l - 1:
                    yTn = sb.tile([P, shard_pad], BF16, tag="yT")
                    nc.scalar.activation(yTn[:], oT[:], func, bias=shift[:], scale=scale[:])
                    npad = shard_pad - g["shard"]
                    if npad > 0:
                        nc.vector.memset(yTn[:, g["shard"]:], 0.0)
                    yT = yTn
                else:
                    nc.scalar.activation(yF[:], oT[:], func, bias=shift[:], scale=scale[:])
                    nc.sync.dma_start(out_t[:], yF[:])

    nc.compile()
    return nc


# ----------------------------------------------------------------------------
# entry point
# ----------------------------------------------------------------------------

def _make_layers(params):
    layers = []
    for (W, asr, ads, gmm, bet) in params:
        H, C = asr.shape
        HC = H * C
        R = ((HC + H) + 127) // 128 * 128   # bf16 elems, 256B-aligned rows
        layers.append({"H": H, "C": C, "R": R, "hs_off": HC})
    return layers


def _make_inputs(g, layers, params, x, in_dim):
    wexts = []
    for (W, asr, ads, gmm, bet), L in zip(params, layers):
        H, C = L["H"], L["C"]
        w_s = np.einsum("khc,hc->kh", W.reshape(W.shape[0], H, C), asr)
        w_d = np.einsum("khc,hc->kh", W.reshape(W.shape[0], H, C), ads)
        wexts.append(np.concatenate([W, w_s, w_d], axis=1).astype(BF))

    gbm = np.zeros((P, 2 * len(layers)), np.float32)
    for l, (W, asr, ads, gmm, bet) in enumerate(params):
        gbm[:len(gmm), 2 * l] = gmm
        gbm[:len(bet), 2 * l + 1] = bet

    shard, shard_pad = g["shard"], g["shard_pad"]
    in_maps = []
    for c in range(NCORES):
        nodes = g["out_nodes"][c]
        xT_c = np.zeros((in_dim, shard_pad), BF)
        xT_c[:, :shard] = x[nodes].T.astype(BF)
        spad = np.zeros((P, 1), BF)
        lastbase = (g["ntiles"] - 1) * P
        for p in range(P):
            if lastbase + p >= shard:
                spad[p, 0] = BF(-1e30)
        m = {"xT": xT_c, "idx": np.ascontiguousarray(g["idx16"][c]),
             "gb": gbm, "spad": spad}
        for l, w in enumerate(wexts):
            m[f"wext{l}"] = w
        in_maps.append(m)
    return in_maps


def build_for_inputs(x, edge_index, params_list, ablate=(), nlayers=3):
    """Build (nc, in_maps) without running."""
    x = np.asarray(x, np.float32)
    N, in_dim = x.shape
    g = _prep(np.asarray(edge_index), N)
    params = params_list[:nlayers]
    layers = _make_layers(params)
    nc = _build_program(g, layers, in_dim, ablate=ablate)
    in_maps = _make_inputs(g, layers, params, x, in_dim)
    return nc, in_maps, g, layers


def kernel(x, edge_index,
           W0, a_src0, a_dst0, b0, gamma0, beta0,
           W1, a_src1, a_dst1, b1, gamma1, beta1,
           W2, a_src2, a_dst2, b2, gamma2, beta2, _profile=None, _nlayers=3):
    x = np.asarray(x, np.float32)
    N, in_dim = x.shape
    g = _prep(np.asarray(edge_index), N)

    params = [(np.asarray(W0, np.float32), np.asarray(a_src0, np.float32), np.asarray(a_dst0, np.float32),
               np.asarray(gamma0, np.float32), np.asarray(beta0, np.float32)),
              (np.asarray(W1, np.float32), np.asarray(a_src1, np.float32), np.asarray(a_dst1, np.float32),
               np.asarray(gamma1, np.float32), np.asarray(beta1, np.float32)),
              (np.asarray(W2, np.float32), np.asarray(a_src2, np.float32), np.asarray(a_dst2, np.float32),
               np.asarray(gamma2, np.float32), np.asarray(beta2, np.float32))][:_nlayers]

    layers = _make_layers(params)
    nc = _build_program(g, layers, in_dim)
    in_maps = _make_inputs(g, layers, params, x, in_dim)

    if _profile is not None:
        _profile["nc"] = nc
        _profile["in_maps"] = in_maps
    res = bass_utils.run_bass_kernel_spmd(nc, in_maps, core_ids=list(range(NCORES)))

    C_out = layers[-1]["C"]
    out = np.empty((N, C_out), np.float32)
    for c in range(NCORES):
        yT = res.results[c]["out"]           # [P(feat), shard_pad]
        out[g["out_nodes"][c]] = np.asarray(yT[:C_out, :g["shard"]], np.float32).T
    if _profile is not None:
        _profile["results"] = res
    return out
